# revision 1
# baseline (speedup 1.0000x reference)
"""AdderNet BasicBlock (Adder2D 3x3 + BatchNorm(train) + ReLU) on 8 TRN2 cores.

Problem: x[4,64,32,32], weight[64,64,3,3], gamma[64], beta[64] ->
    out[b,o,y,x] = relu(BN_train(-sum_{c,ky,kx} |x_pad[b,c,y+ky,x+kx] - w[o,c,ky,kx]|))

Sharding: output channels O=64 split 8 per core. BatchNorm stats are per-channel
over (B,H,W), so each core's 8 channels are fully self-contained: no collectives.

Per-core dataflow (all shapes hardcoded):
  - x held in SBUF as XP[128, 2*34*34] f32: partition p = (h, c) with h=p//64,
    c=p%64; free (u, y, x) holds batch b = 2*u + h, zero-padded spatial. One
    strided view covers all 4 batches at F=2048 per (output-channel o, tap).
    f16 copies (plus a 1-element-shifted one for odd tap offsets, keeping APs
    4-byte aligned) feed the DVE 4x-mode path.
  - For each (o, tap): D[128, 2, 32, 32] f16 = |XP_view - w[o, c, tap]|:
    ~30 taps on ACT (activation Abs with per-partition bias=-w, one fused op),
    the rest on DVE (tensor_scalar subtract at 4x, then sign-bit clear via
    tensor_scalar bitwise_and 0x7FFF on the u16 view, also 4x). abs_max and
    tensor_tensor_reduce are rejected/broken in HW ISA; this chain is exact.
  - PE reduces over partitions, accumulating all 8 channels x 9 taps into one
    persistent PSUM tile S[32, 1024]. lhsT is a one-hot f16 selector
    msel(o,u)[128, 32] mapping partition half h to output row o*4 + 2u + h, so
    every matmul writes base partition 0 (hardware constraint) and rows for
    other channels just accumulate zeros. f16 matmuls measured 221.6 ns at
    N=512 (f32r pays +45ns/matmul on weight load; fp32 is 4x slower).
  - Epilogue from PSUM: per-channel mean via free-reduce + tiny selector matmul;
    centered subtract (doubles as the PSUM drain), ACT Square+accum for var,
    out = relu((S-m)*A + beta), single DMA out [32,1024] = (o_local*4+b, y*32+x).
  - Measured on HW: main loop ~66-70 us/core, plus ~29 us fixed (input DMA,
    BN epilogue, Tile tail barrier); rel RMS error ~5e-5 (f16 D quantization).

kernel() is self-contained: builds the Bass program once, shards inputs on host,
runs via bass_utils.run_bass_kernel_spmd on cores 0..7, reassembles full output.
"""

import functools
import os

import numpy as np

B, C, O, H, W = 4, 64, 64, 32, 32
K, PAD = 3, 1
HP, WP = H + 2 * PAD, W + 2 * PAD  # 34, 34
L = H * W  # 1024
SPP = HP * WP  # 1156 padded spatial per batch
NCORES = 8
O_PER = O // NCORES  # 8
NB2 = B // 2  # bpairs
EPS = 1e-5
NSTAT = O_PER * B  # 32 rows of S
NPIX = B * L  # 4096 values per channel for BN stats

# absdiff engine split: 72 ops per core (8 o x 9 taps). ACT is a bit faster per op
# ((F+222)*0.83 vs (F+58)*1.04 ns), so give it more.
N_ACT_OPS = int(os.environ.get("KRN_ACT_OPS", "32"))
N_GPS_OPS = int(os.environ.get("KRN_GPS_OPS", "0"))  # gpsimd subtract is ~10x too slow on HW
N_PRESUM = int(os.environ.get("KRN_PRESUM", "0"))  # only pays once producers beat PE
D_BUFS = int(os.environ.get("KRN_D_BUFS", "10"))
MM_REPEAT = int(os.environ.get("KRN_MM_REPEAT", "1"))  # bench-only: scales PE work


def _engine_schedule(n_ops: int):
    """Return list of 'v'/'a'/'g' of length n_ops, interleaving engines evenly."""
    n_g = min(N_GPS_OPS, n_ops)
    n_a = min(N_ACT_OPS, n_ops - n_g)
    n_v = n_ops - n_a - n_g
    counts = {"v": n_v, "a": n_a, "g": n_g}
    acc = {k: 0.0 for k in counts}
    sched = []
    for _ in range(n_ops):
        for k in counts:
            acc[k] += counts[k] / n_ops
        pick = max(acc, key=lambda k: acc[k])
        acc[pick] -= 1.0
        sched.append(pick)
    return sched




def _emit_main(nc, tc, mybir, xp4, xph4, xpho4, wcols, nwcols, mselh, dpool, ps, sched):
    from concourse import mybir as _mb

    f16 = _mb.dt.float16
    u16 = _mb.dt.uint16

    def absdiff(o, tap):
        """Emit |x - w| for (o, tap) -> f16 tile d[128, NB2, H, W]."""
        ky, kx = tap // 3, tap % 3
        idx = o * 9 + tap
        eng = sched[idx]
        if eng == "a":
            view = xp4[:, :, ky : ky + H, kx : kx + W]
            d = dpool.tile([128, NB2, H, W], f16, tag="da", name=f"da{idx}")
            nc.scalar.activation(
                out=d[:], in_=view, func=_mb.ActivationFunctionType.Abs,
                bias=nwcols[:, idx : idx + 1], scale=1.0,
            )
            return d
        if kx == 1:
            view = xpho4[:, :, ky : ky + H, kx - 1 : kx - 1 + W]
        else:
            view = xph4[:, :, ky : ky + H, kx : kx + W]
        d1 = dpool.tile([128, NB2, H, W], f16, tag="d1", name=f"d1_{idx}")
        if eng == "g":
            nc.gpsimd.tensor_scalar_sub(d1[:], view, wcols[:, idx : idx + 1])
        else:
            nc.vector.tensor_scalar_sub(d1[:], view, wcols[:, idx : idx + 1])
        d = dpool.tile([128, NB2, H, W], f16, tag="dv", name=f"dv{idx}")
        nc.vector.tensor_scalar(
            out=d[:].bitcast(u16), in0=d1[:].bitcast(u16),
            scalar1=0x7FFF, scalar2=None, op0=_mb.AluOpType.bitwise_and,
        )
        return d

    first = [True, True]

    def mm(o, d, last):
        d2 = d.rearrange("p u a b -> p (u a b)")
        for rep in range(MM_REPEAT):
            for u in range(NB2):
                for half in range(2):
                    nc.tensor.matmul(
                        ps[half][:, :],
                        lhsT=mselh[:, (o * 2 + u) * NSTAT : (o * 2 + u + 1) * NSTAT],
                        rhs=d2[:, u * L + half * 512 : u * L + half * 512 + 512],
                        start=first[half],
                        stop=(last and rep == MM_REPEAT - 1 and u == NB2 - 1),
                    )
                    first[half] = False

    for o in range(O_PER):
        taps = list(range(9))
        # pick one presum pair per channel for the first N_PRESUM channels:
        # two non-ACT taps whose |diff| tiles get added on DVE before PE.
        pair = None
        if o < N_PRESUM:
            cand = [t for t in taps if sched[o * 9 + t] != "a"]
            if len(cand) >= 2:
                pair = (cand[0], cand[1])
        last_of_o = o == O_PER - 1
        if pair is not None:
            di = absdiff(o, pair[0])
            dj = absdiff(o, pair[1])
            dsum = dpool.tile([128, NB2, H, W], f16, tag="dsum", name=f"dsum{o}")
            nc.vector.tensor_add(dsum[:], di[:], dj[:])
            mm(o, dsum, False)
        rest = [t for t in taps if pair is None or t not in pair]
        for i, tap in enumerate(rest):
            d = absdiff(o, tap)
            mm(o, d, last_of_o and i == len(rest) - 1)

@functools.lru_cache(maxsize=4)
def _build_program(bench_iters=0):
    from contextlib import ExitStack

    import concourse.tile as tile
    from concourse import bacc, mybir

    f32 = mybir.dt.float32
    f32r = mybir.dt.float32r
    f16 = mybir.dt.float16
    u16 = mybir.dt.uint16

    nc = bacc.Bacc("TRN2", target_bir_lowering=False, debug=False)

    x_t = nc.dram_tensor("x", (B, C, H, W), f32, kind="ExternalInput")
    # wpack[:, :72] = wcols (w[o_g, p%64, tap]), [:, 72:144] = -wcols
    wpack_t = nc.dram_tensor("wpack", (128, 2 * O_PER * 9), f32, kind="ExternalInput")
    # mselh[p, (o*2+u)*32 + j] = 1.0 iff j == o*4 + 2u + p//64
    mselh_t = nc.dram_tensor("mselh", (128, O_PER * 2 * NSTAT), f16, kind="ExternalInput")
    # spack[:, :32] = osel, [:, 32] = -gamma col, [:, 33] = beta col
    spack_t = nc.dram_tensor("spack", (NSTAT, NSTAT + 2), f32, kind="ExternalInput")
    out_t = nc.dram_tensor("out", (NSTAT, L), f32, kind="ExternalOutput")

    sched = _engine_schedule(O_PER * 9)

    with tile.TileContext(nc) as tc, ExitStack() as ctx:
        consts = ctx.enter_context(tc.tile_pool(name="consts", bufs=1))
        dpool = ctx.enter_context(tc.tile_pool(name="dpool", bufs=D_BUFS))
        spool = ctx.enter_context(tc.tile_pool(name="spool", bufs=2))
        psum_main = ctx.enter_context(tc.tile_pool(name="psum_main", bufs=1, space="PSUM"))
        psum_stat = ctx.enter_context(tc.tile_pool(name="psum_stat", bufs=2, space="PSUM"))

        # ---- constants / inputs to SBUF ----
        wpack = consts.tile([128, 2 * O_PER * 9], f32)
        mselh = consts.tile([128, O_PER * 2 * NSTAT], f16)
        spack = consts.tile([NSTAT, NSTAT + 2], f32)
        nc.sync.dma_start(out=wpack[:], in_=wpack_t[:, :])
        nc.sync.dma_start(out=mselh[:], in_=mselh_t[:, :])
        nc.sync.dma_start(out=spack[:], in_=spack_t[:, :])
        wcols = wpack[:, 0 : O_PER * 9]
        nwcols = wpack[:, O_PER * 9 : 2 * O_PER * 9]
        osel = spack[:, 0:NSTAT]
        gcol = spack[:, NSTAT : NSTAT + 1]
        bcol = spack[:, NSTAT + 1 : NSTAT + 2]

        # ---- padded input: XP[128, 2*1156], partition=(b_half, c), free=(bpair, y, x)
        xp = consts.tile([128, NB2 * SPP], f32)
        xp4 = xp.rearrange("p (u a b) -> p u a b", u=NB2, a=HP, b=WP)
        for u in range(NB2):
            nc.gpsimd.memset(xp4[:, u, 0, :], 0.0)
            nc.gpsimd.memset(xp4[:, u, HP - 1, :], 0.0)
            nc.gpsimd.memset(xp4[:, u, :, 0], 0.0)
            nc.gpsimd.memset(xp4[:, u, :, WP - 1], 0.0)
        for b in range(B):
            h, u = b % 2, b // 2
            nc.sync.dma_start(
                out=xp4[h * 64 : h * 64 + 64, u, PAD : PAD + H, PAD : PAD + W],
                in_=x_t[b, :, :, :],
            )

        # f16 copy of the padded input for the 4x-mode DVE chain
        xph = consts.tile([128, NB2 * SPP], f16)
        nc.vector.tensor_copy(out=xph[:], in_=xp[:])
        xph4 = xph.rearrange("p (u a b) -> p u a b", u=NB2, a=HP, b=WP)
        # +1-element shifted copy: keeps the kx==1 taps 4-byte aligned for 4x mode
        xpho = consts.tile([128, NB2 * SPP], f16)
        nc.scalar.copy(out=xpho[:, 0 : NB2 * SPP - 1], in_=xph[:, 1 : NB2 * SPP])
        xpho4 = xpho.rearrange("p (u a b) -> p u a b", u=NB2, a=HP, b=WP)

        # ---- main loop: S[o*4+b, l] accumulates over taps in two PSUM halves ----
        ps_big = psum_main.tile([NSTAT, 2 * 512], f32, name="ps_big")
        ps = [ps_big[:, h * 512 : h * 512 + 512] for h in range(2)]
        import contextlib

        loop_cm = (
            tc.For_i(0, bench_iters, 1) if bench_iters else contextlib.nullcontext()
        )
        with loop_cm:
            _emit_main(nc, tc, mybir, xp4, xph4, xpho4, wcols, nwcols, mselh, dpool, ps, sched)

        # ---- epilogue: BN stats + normalize + relu ----
        # per-row sums over l, then per-channel (replicated) via selector matmul
        sums = spool.tile([NSTAT, 1], f32, tag="small1")
        nc.vector.tensor_reduce(
            out=sums[:], in_=ps_big[:], axis=mybir.AxisListType.X,
            op=mybir.AluOpType.add,
        )
        sum_ps = psum_stat.tile([NSTAT, 1], f32, tag="statps")
        nc.tensor.matmul(
            sum_ps[:], lhsT=osel, rhs=sums[:],
            start=True, stop=True,
        )
        m32 = spool.tile([NSTAT, 1], f32, tag="small2")
        nc.vector.tensor_scalar_mul(m32[:], sum_ps[:], 1.0 / NPIX)

        # centered values (also drains PSUM -> SBUF)
        dctr = spool.tile([NSTAT, L], f32, tag="big")
        nc.vector.tensor_scalar_sub(dctr[:], ps_big[:], m32[:])
        scr = spool.tile([NSTAT, L], f32, tag="big2")
        sqs = spool.tile([NSTAT, 1], f32, tag="small3")
        nc.scalar.activation(
            out=scr[:], in_=dctr[:], func=mybir.ActivationFunctionType.Square,
            accum_out=sqs[:],
        )
        var_ps = psum_stat.tile([NSTAT, 1], f32, tag="statps2")
        nc.tensor.matmul(
            var_ps[:], lhsT=osel, rhs=sqs[:],
            start=True, stop=True,
        )
        # std = sqrt(var/NPIX + eps); rinv = 1/std
        epscol = spool.tile([NSTAT, 1], f32, tag="eps")
        nc.vector.memset(epscol[:], EPS)
        std32 = spool.tile([NSTAT, 1], f32, tag="small4")
        nc.scalar.activation(
            out=std32[:], in_=var_ps[:], func=mybir.ActivationFunctionType.Sqrt,
            bias=epscol[:], scale=1.0 / NPIX,
        )
        rinv = spool.tile([NSTAT, 1], f32, tag="small5")
        nc.vector.reciprocal(rinv[:], std32[:])
        # A = -gamma*rinv ; out = relu((S - m)*A + beta)
        acol = spool.tile([NSTAT, 1], f32, tag="small7")
        nc.vector.tensor_mul(acol[:], gcol, rinv[:])

        outf = spool.tile([NSTAT, L], f32, tag="outf")
        nc.scalar.activation(
            out=outf[:], in_=dctr[:], func=mybir.ActivationFunctionType.Relu,
            bias=bcol, scale=acol[:],
        )
        nc.sync.dma_start(out=out_t[:, :], in_=outf[:])

    nc.compile()
    return nc


def _host_inputs(x, weight, gamma, beta):
    """Build the 8 per-core input maps."""
    x = np.ascontiguousarray(x, dtype=np.float32)
    weight = np.asarray(weight, dtype=np.float32)
    gamma = np.asarray(gamma, dtype=np.float32)
    beta = np.asarray(beta, dtype=np.float32)

    msel = np.zeros((128, O_PER * 2 * NSTAT), dtype=np.float32)
    for o in range(O_PER):
        for u in range(NB2):
            for p_half in range(2):
                j = o * 4 + 2 * u + p_half
                col = (o * 2 + u) * NSTAT + j
                msel[p_half * 64 : (p_half + 1) * 64, col] = 1.0
    osel = np.zeros((NSTAT, NSTAT), dtype=np.float32)
    for p in range(NSTAT):
        for m in range(NSTAT):
            if p // B == m // B:
                osel[p, m] = 1.0

    in_maps = []
    for core in range(NCORES):
        osl = slice(core * O_PER, (core + 1) * O_PER)
        w = weight[osl]  # [8, 64, 3, 3]
        # wcols[p, o*9+tap] = w[o, p%64, tap//3, tap%3]
        wc = w.reshape(O_PER, C, 9).transpose(1, 0, 2).reshape(C, O_PER * 9)
        wcols = np.concatenate([wc, wc], axis=0).astype(np.float32)  # [128, 72]
        wpack = np.concatenate([wcols, -wcols], axis=1)  # [128, 144]
        # gcol[p] = -gamma[o(p)] with o = p//4 (A = -gamma*rinv)
        gcol = np.repeat(-gamma[osl], B).reshape(NSTAT, 1).astype(np.float32)
        bcol = np.repeat(beta[osl], B).reshape(NSTAT, 1).astype(np.float32)
        spack = np.concatenate([osel, gcol, bcol], axis=1)  # [32, 34]
        in_maps.append(
            {
                "x": x,
                "wpack": np.ascontiguousarray(wpack),
                "mselh": msel.astype(np.float16),
                "spack": np.ascontiguousarray(spack),
            }
        )
    return in_maps


def _assemble(results):
    out = np.empty((B, O, H, W), dtype=np.float32)
    for core, res in enumerate(results):
        arr = res["out"].reshape(O_PER, B, H, W)  # row = o*4+b
        out[:, core * O_PER : (core + 1) * O_PER] = arr.transpose(1, 0, 2, 3)
    return out


def kernel(x, weight, gamma, beta, _trace=False):
    from concourse import bass_utils

    nc = _build_program()
    in_maps = _host_inputs(x, weight, gamma, beta)
    res = bass_utils.run_bass_kernel_spmd(
        nc, in_maps, core_ids=list(range(NCORES)), trace=_trace
    )
    out = _assemble(res.results)
    if _trace:
        return out, res
    return out



# revision 18
# speedup vs baseline: 1.4583x; 1.4583x over previous
"""AdderNet BasicBlock (Adder2D 3x3 + BatchNorm(train) + ReLU) on 8 TRN2 cores.

Problem: x[4,64,32,32], weight[64,64,3,3], gamma[64], beta[64] ->
    out[b,o,y,x] = relu(BN_train(-sum_{c,ky,kx} |x_pad[b,c,y+ky,x+kx] - w[o,c,ky,kx]|))

Sharding: output channels O=64 split 8 per core. BatchNorm stats are per-channel
over (B,H,W), so each core's 8 channels are fully self-contained: no collectives.

Key algebraic trick (mode "relu", default): |z| = 2*relu(z) - z, so
  S_true[o,b,y,x] = sum_{c,t} |x_t - w_t|
                  = 2*sum_{c,t} relu(x_t - w_t) - XBOX[b,y,x] + WS[o]
where XBOX = 3x3 box-sum of x over c (o-independent: computed once per core)
and WS[o] = sum w[o] (a per-channel constant that CANCELS in BatchNorm -
shift invariance - so it is simply dropped). relu(x-w) is ONE fused DVE op
(tensor_scalar op0=subtract op1=max scalar2=0, 4x mode, HW-validated) vs the
2-op sub+mask chain |x-w| needs; the *2 is folded into the PE selector values
and XBOX is accumulated into the same PSUM with a -1-valued selector.

Per-core dataflow (all shapes hardcoded):
  - Host supplies x as TWO padded f16 images XPH/XPHO[128, 2*34*34]:
    partition p=(h,c) with h=p//64, c=p%64; free (u,y,x) holds batch b=2u+h,
    zero-padded spatial. XPHO is XPH shifted one element (keeps kx==1 tap
    views 4-byte aligned for the DVE 4x mode). Two big contiguous DMAs.
  - XBOX passes first on PE (fills PE ramp-up): rhs = raw xph tap views
    (KRN_XBOX=9) or a DVE-presummed vertical sum ver (KRN_XBOX=3: 2 adds +
    3 passes), lhsT = -1-valued selector bsel_u broadcasting to all o rows.
  - For each (o, tap): R[128, 2, 32, 32] f16 = relu(x - w): ~27 taps on ACT
    (activation Relu with bias=-w), the rest fused on DVE (594ns).
  - ~21 presum pairs: two R tiles added on DVE so one PE pass covers two
    taps.
  - PE accumulates everything into one PSUM tile S[32, 1024] f32
    (row = o_local*4 + b) holding S_eff = 2*P1 - XBOX = S_true - WS.
  - Epilogue: a dummy Sqrt at kernel head pins activation table 3 so no
    mid-kernel LoadActFuncSet. Per-row sums via free-reduce + tiny selector
    matmul -> mean; N*var in ONE ACT op (Square with bias=-mean, accum_out);
    mean folded into the final activation bias:
    out = relu(S*A + (beta - m*A)), A = -gamma*rstd, emitted in 2
    column-halves so the out DMA overlaps the second Relu.

kernel() is self-contained: builds the Bass program once, shards inputs on host,
runs via bass_utils.run_bass_kernel_spmd on cores 0..7, reassembles full output.
"""

import functools
import os

import numpy as np

B, C, O, H, W = 4, 64, 64, 32, 32
K, PAD = 3, 1
HP, WP = H + 2 * PAD, W + 2 * PAD  # 34, 34
L = H * W  # 1024
SPP = HP * WP  # 1156 padded spatial per batch
NCORES = 8
O_PER = O // NCORES  # 8
NB2 = B // 2  # bpairs
EPS = 1e-5
NSTAT = O_PER * B  # 32 rows of S
NPIX = B * L  # 4096 values per channel for BN stats

MODE = os.environ.get("KRN_MODE", "relu")  # "relu" (|z|=2relu(z)-z) or "abs"
N_ACT_OPS = int(os.environ.get("KRN_ACT_OPS", "26"))  # taps on ACT
N_PRESUM = int(os.environ.get("KRN_PRESUM", "21"))  # pair-adds (each saves a PE pass)
D_BUFS = int(os.environ.get("KRN_D_BUFS", "12"))
XBOX_PASSES = int(os.environ.get("KRN_XBOX", "9"))  # 9: raw; 3: ver-presum + 3


def _engine_schedule(n_ops: int):
    """Return list of 'v'/'a' of length n_ops, interleaving engines evenly."""
    n_a = min(N_ACT_OPS, n_ops)
    counts = {"v": n_ops - n_a, "a": n_a}
    acc = {k: 0.0 for k in counts}
    sched = []
    for _ in range(n_ops):
        for k in counts:
            acc[k] += counts[k] / n_ops
        pick = max(acc, key=lambda k: acc[k])
        acc[pick] -= 1.0
        sched.append(pick)
    return sched


def _presum_plan():
    """pairs per o-channel: distribute N_PRESUM pair-adds round-robin, max 4/o."""
    per_o = [0] * O_PER
    left = min(N_PRESUM, 4 * O_PER)
    i = 0
    while left > 0:
        if per_o[i % O_PER] < 4:
            per_o[i % O_PER] += 1
            left -= 1
        i += 1
    return per_o


def _emit_main(nc, tc, mybir, xph4, xpho4, wcols, nwcols, mselh, bsel, dpool, ps, sched):
    from concourse import mybir as _mb

    f16 = _mb.dt.float16
    u16 = _mb.dt.uint16
    first = [True, True]

    def mm4(lhsT_of_u, view4, last):
        """One PE pass: 4 matmuls (u x psum-half) of a [128,2,32,32] view."""
        for u in range(NB2):
            for half in range(2):
                nc.tensor.matmul(
                    ps[half][:, :],
                    lhsT=lhsT_of_u(u),
                    rhs=view4[:, u, half * 16 : half * 16 + 16, :],
                    start=first[half],
                    stop=(last and u == NB2 - 1),
                )
                first[half] = False

    # ---- XBOX: -sum_{c,taps} x_window accumulated into all rows ----
    if MODE == "relu":
        bbase = O_PER * 2 * NSTAT
        bsel_of_u = lambda u: bsel[:, bbase + u * NSTAT : bbase + (u + 1) * NSTAT]
        if XBOX_PASSES == 9:
            for tap in range(9):
                ky, kx = tap // 3, tap % 3
                view = xph4[:, :, ky : ky + H, kx : kx + W]
                mm4(bsel_of_u, view, False)
        else:  # ver presum: ver[y][x'] = sum_ky xph[y+ky][x'], then 3 kx passes
            vtmp = dpool.tile([128, NB2, H, WP], f16, tag="vtmp", name="vtmp")
            nc.vector.tensor_add(
                vtmp[:], xph4[:, :, 0:H, :], xph4[:, :, 1 : 1 + H, :]
            )
            ver = dpool.tile([128, NB2, H, WP], f16, tag="ver", name="ver")
            nc.vector.tensor_add(ver[:], vtmp[:], xph4[:, :, 2 : 2 + H, :])
            ver4 = ver[:]
            for kx in range(3):
                mm4(bsel_of_u, ver4[:, :, :, kx : kx + W], False)

    def rdiff(o, tap):
        """Emit relu(x-w) (mode relu) or |x-w| (mode abs) -> f16 tile."""
        ky, kx = tap // 3, tap % 3
        idx = o * 9 + tap
        eng = sched[idx]
        if eng == "a":
            view = xph4[:, :, ky : ky + H, kx : kx + W]
            d = dpool.tile([128, NB2, H, W], f16, tag="da", name=f"da{idx}")
            func = (
                _mb.ActivationFunctionType.Relu
                if MODE == "relu"
                else _mb.ActivationFunctionType.Abs
            )
            nc.scalar.activation(
                out=d[:], in_=view, func=func,
                bias=nwcols[:, idx : idx + 1], scale=1.0,
            )
            return d
        if kx == 1:
            view = xpho4[:, :, ky : ky + H, kx - 1 : kx - 1 + W]
        else:
            view = xph4[:, :, ky : ky + H, kx : kx + W]
        if MODE == "relu":
            d = dpool.tile([128, NB2, H, W], f16, tag="dv", name=f"dv{idx}")
            nc.vector.tensor_scalar(
                out=d[:], in0=view, scalar1=wcols[:, idx : idx + 1], scalar2=0.0,
                op0=_mb.AluOpType.subtract, op1=_mb.AluOpType.max,
            )
            return d
        d1 = dpool.tile([128, NB2, H, W], f16, tag="d1", name=f"d1_{idx}")
        nc.vector.tensor_scalar_sub(d1[:], view, wcols[:, idx : idx + 1])
        d = dpool.tile([128, NB2, H, W], f16, tag="dv", name=f"dv{idx}")
        nc.vector.tensor_scalar(
            out=d[:].bitcast(u16), in0=d1[:].bitcast(u16),
            scalar1=0x7FFF, scalar2=None, op0=_mb.AluOpType.bitwise_and,
        )
        return d

    per_o_pairs = _presum_plan()
    for o in range(O_PER):
        p = per_o_pairs[o]
        # engine-aware pairing: pair ACT taps with DVE taps so one pass's
        # producer latency mixes a slow (1892ns) and fast (594ns) op.
        taps = list(range(9))
        a_taps = [t for t in taps if sched[o * 9 + t] == "a"]
        v_taps = [t for t in taps if sched[o * 9 + t] == "v"]
        items = []
        for i in range(p):
            if a_taps and v_taps:
                items.append((v_taps.pop(0), a_taps.pop(0)))
            elif len(v_taps) >= 2:
                items.append((v_taps.pop(0), v_taps.pop(0)))
            else:
                items.append((a_taps.pop(0), a_taps.pop(0)))
        # singles: alternate engines to smooth PE feed
        singles = []
        while a_taps or v_taps:
            if v_taps:
                singles.append((v_taps.pop(0),))
            if a_taps:
                singles.append((a_taps.pop(0),))
        items += singles
        last_of_o = o == O_PER - 1
        for i, item in enumerate(items):
            last = last_of_o and i == len(items) - 1
            if len(item) == 2:
                d0 = rdiff(o, item[0])
                d1 = rdiff(o, item[1])
                dsum = dpool.tile(
                    [128, NB2, H, W], _mb.dt.float16, tag="dsum", name=f"ds{o}_{i}"
                )
                nc.vector.tensor_add(dsum[:], d0[:], d1[:])
                dmm = dsum
            else:
                dmm = rdiff(o, item[0])
            sel_of_u = lambda u: mselh[
                :, (o * 2 + u) * NSTAT : (o * 2 + u + 1) * NSTAT
            ]
            if not last:
                for u in range(NB2):
                    for half in range(2):
                        nc.tensor.matmul(
                            ps[half][:, :], lhsT=sel_of_u(u),
                            rhs=dmm[:][:, u, half * 16 : half * 16 + 16, :],
                            start=first[half], stop=False,
                        )
                        first[half] = False
            else:
                # final pass: close bank 0 first so its stats reduce can
                # overlap the remaining bank-1 matmuls
                for half in range(2):
                    for u in range(NB2):
                        nc.tensor.matmul(
                            ps[half][:, :], lhsT=sel_of_u(u),
                            rhs=dmm[:][:, u, half * 16 : half * 16 + 16, :],
                            start=first[half], stop=(u == NB2 - 1),
                        )
                        first[half] = False


@functools.lru_cache(maxsize=4)
def _build_program(bench_iters=0):
    import contextlib
    from contextlib import ExitStack

    import concourse.tile as tile
    from concourse import bacc, mybir

    f32 = mybir.dt.float32
    f16 = mybir.dt.float16

    nc = bacc.Bacc("TRN2", target_bir_lowering=False, debug=False)

    # padded f16 input images (see module docstring for layout)
    xph_t = nc.dram_tensor("xph", (128, NB2 * SPP), f16, kind="ExternalInput")
    xpho_t = nc.dram_tensor("xpho", (128, NB2 * SPP), f16, kind="ExternalInput")
    # wpack[:, :72] = wcols (w[o_g, p%64, tap]), [:, 72:144] = -wcols
    wpack_t = nc.dram_tensor("wpack", (128, 2 * O_PER * 9), f32, kind="ExternalInput")
    # mselh[p, (o*2+u)*32 + j] = msel_scale iff j == o*4 + 2u + p//64;
    # cols 512:576 = bsel_u blocks (-1 at j%4 == 2u + p//64) for XBOX (relu mode)
    mselh_t = nc.dram_tensor("mselh", (128, O_PER * 2 * NSTAT + 2 * NSTAT), f16,
                             kind="ExternalInput")
    # spack[:, :32] = osel, [:, 32] = -gamma col, [:, 33] = beta col
    spack_t = nc.dram_tensor("spack", (NSTAT, NSTAT + 2), f32, kind="ExternalInput")
    out_t = nc.dram_tensor("out", (NSTAT, L), f32, kind="ExternalOutput")

    sched = _engine_schedule(O_PER * 9)

    with tile.TileContext(nc) as tc, ExitStack() as ctx:
        consts = ctx.enter_context(tc.tile_pool(name="consts", bufs=1))
        dpool = ctx.enter_context(tc.tile_pool(name="dpool", bufs=D_BUFS))
        spool = ctx.enter_context(tc.tile_pool(name="spool", bufs=2))
        psum_main = ctx.enter_context(tc.tile_pool(name="psum_main", bufs=1, space="PSUM"))
        psum_stat = ctx.enter_context(tc.tile_pool(name="psum_stat", bufs=2, space="PSUM"))

        # ---- dummy Sqrt first: pins act func table 3 (abs+sqrt+square+relu)
        # so the epilogue triggers no mid-kernel LoadActFuncSet.
        tiny = consts.tile([1, 1], f32)
        nc.vector.memset(tiny[:], 1.0)
        tiny2 = consts.tile([1, 1], f32)
        nc.scalar.activation(
            out=tiny2[:], in_=tiny[:], func=mybir.ActivationFunctionType.Sqrt
        )

        # ---- inputs to SBUF; spread across issue queues to overlap ----
        xph = consts.tile([128, NB2 * SPP], f16)
        xpho = consts.tile([128, NB2 * SPP], f16)
        wpack = consts.tile([128, 2 * O_PER * 9], f32)
        mselh = consts.tile([128, O_PER * 2 * NSTAT + 2 * NSTAT], f16)
        spack = consts.tile([NSTAT, NSTAT + 2], f32)
        # halves complete ~0.9us apart so the first XBOX/tap work starts early
        HSPP = NB2 * SPP // 2
        nc.gpsimd.dma_start(out=mselh[:], in_=mselh_t[:, :])
        nc.sync.dma_start(out=xph[:, 0:HSPP], in_=xph_t[:, 0:HSPP])
        nc.sync.dma_start(out=xph[:, HSPP:], in_=xph_t[:, HSPP:])
        nc.gpsimd.dma_start(out=xpho[:, 0:HSPP], in_=xpho_t[:, 0:HSPP])
        nc.gpsimd.dma_start(out=xpho[:, HSPP:], in_=xpho_t[:, HSPP:])
        nc.gpsimd.dma_start(out=wpack[:], in_=wpack_t[:, :])
        nc.sync.dma_start(out=spack[:], in_=spack_t[:, :])
        wcols = wpack[:, 0 : O_PER * 9]
        nwcols = wpack[:, O_PER * 9 : 2 * O_PER * 9]
        msel = mselh[:, 0 : O_PER * 2 * NSTAT]
        bsel = None
        osel = spack[:, 0:NSTAT]
        gcol = spack[:, NSTAT : NSTAT + 1]
        bcol = spack[:, NSTAT + 1 : NSTAT + 2]

        xph4 = xph.rearrange("p (u a b) -> p u a b", u=NB2, a=HP, b=WP)
        xpho4 = xpho.rearrange("p (u a b) -> p u a b", u=NB2, a=HP, b=WP)

        # ---- main loop: S[o*4+b, l] accumulates over taps in two PSUM banks.
        # Separate tiles (not slices of one) so bank-0 readers need not wait
        # for bank-1's final matmuls (tile-granularity dependency tracking).
        ps0 = psum_main.tile([NSTAT, 512], f32, name="ps0", tag="ps0")
        ps1 = psum_main.tile([NSTAT, 512], f32, name="ps1", tag="ps1")
        ps = [ps0, ps1]

        loop_cm = (
            tc.For_i(0, bench_iters, 1) if bench_iters else contextlib.nullcontext()
        )
        with loop_cm:
            _emit_main(
                nc, tc, mybir, xph4, xpho4, wcols, nwcols, msel, mselh, dpool, ps, sched
            )

        # ---- epilogue: BN stats + normalize + relu ----
        # per-row sums over l: bank-0 on DVE overlaps bank-1 on ACT (Copy+accum)
        sums0 = spool.tile([NSTAT, 1], f32, tag="small0")
        nc.vector.tensor_reduce(
            out=sums0[:], in_=ps0[:], axis=mybir.AxisListType.X,
            op=mybir.AluOpType.add,
        )
        junk1 = spool.tile([NSTAT, 512], f16, tag="junk1")
        sums1 = spool.tile([NSTAT, 1], f32, tag="small1")
        nc.scalar.activation(
            out=junk1[:], in_=ps1[:], func=mybir.ActivationFunctionType.Copy,
            accum_out=sums1[:],
        )
        sums = spool.tile([NSTAT, 1], f32, tag="small1b")
        nc.vector.tensor_add(sums[:], sums0[:], sums1[:])
        sum_ps = psum_stat.tile([NSTAT, 1], f32, tag="statps")
        nc.tensor.matmul(sum_ps[:], lhsT=osel, rhs=sums[:], start=True, stop=True)
        mneg = spool.tile([NSTAT, 1], f32, tag="small2")
        nc.vector.tensor_scalar_mul(mneg[:], sum_ps[:], -1.0 / NPIX)

        # N*var = sum_l (S - m)^2: per-bank ACT Square(S + (-m)) with accum_out
        junk = spool.tile([NSTAT, 512], f16, tag="junk")
        nv0 = spool.tile([NSTAT, 1], f32, tag="small3")
        nv1 = spool.tile([NSTAT, 1], f32, tag="small3b")
        nc.scalar.activation(
            out=junk[:], in_=ps0[:], func=mybir.ActivationFunctionType.Square,
            bias=mneg[:], scale=1.0, accum_out=nv0[:],
        )
        nc.scalar.activation(
            out=junk[:], in_=ps1[:], func=mybir.ActivationFunctionType.Square,
            bias=mneg[:], scale=1.0, accum_out=nv1[:],
        )
        nvar = spool.tile([NSTAT, 1], f32, tag="small3c")
        nc.vector.tensor_add(nvar[:], nv0[:], nv1[:])
        nvar_ps = psum_stat.tile([NSTAT, 1], f32, tag="statps2")
        nc.tensor.matmul(nvar_ps[:], lhsT=osel, rhs=nvar[:], start=True, stop=True)

        # std = sqrt(nvar/NPIX + eps); A = -gamma/std; B = beta - m*A
        epscol = spool.tile([NSTAT, 1], f32, tag="eps")
        nc.vector.memset(epscol[:], EPS)
        std = spool.tile([NSTAT, 1], f32, tag="small4")
        nc.scalar.activation(
            out=std[:], in_=nvar_ps[:], func=mybir.ActivationFunctionType.Sqrt,
            bias=epscol[:], scale=1.0 / NPIX,
        )
        rinv = spool.tile([NSTAT, 1], f32, tag="small5")
        nc.vector.reciprocal(rinv[:], std[:])
        acol = spool.tile([NSTAT, 1], f32, tag="small6")
        nc.vector.tensor_mul(acol[:], gcol, rinv[:])
        bfin = spool.tile([NSTAT, 1], f32, tag="small7")
        # bfin = (A * mneg) + beta  (= beta - m*A)
        nc.vector.scalar_tensor_tensor(
            out=bfin[:], in0=acol[:], scalar=mneg[:, 0:1], in1=bcol,
            op0=mybir.AluOpType.mult, op1=mybir.AluOpType.add,
        )

        # out = relu(S*A + bfin), two column-halves so DMA overlaps 2nd relu
        outf = spool.tile([NSTAT, L], f32, tag="outf")
        for hf in range(2):
            cols = slice(hf * 512, hf * 512 + 512)
            nc.scalar.activation(
                out=outf[:, cols], in_=ps[hf][:],
                func=mybir.ActivationFunctionType.Relu,
                bias=bfin, scale=acol[:],
            )
            nc.sync.dma_start(out=out_t[:, cols], in_=outf[:, cols])

    nc.compile()
    return nc


def _host_inputs(x, weight, gamma, beta):
    """Build the 8 per-core input maps."""
    x = np.ascontiguousarray(x, dtype=np.float32)
    weight = np.asarray(weight, dtype=np.float32)
    gamma = np.asarray(gamma, dtype=np.float32)
    beta = np.asarray(beta, dtype=np.float32)

    # padded f16 image: xph[h*64+c, (u, 1+y, 1+x)] = x[2u+h, c, y, x]
    xph = np.zeros((128, NB2, HP, WP), dtype=np.float16)
    for b in range(B):
        h, u = b % 2, b // 2
        xph[h * 64 : (h + 1) * 64, u, PAD : PAD + H, PAD : PAD + W] = x[b]
    xph = xph.reshape(128, NB2 * SPP)
    xpho = np.zeros_like(xph)
    xpho[:, : NB2 * SPP - 1] = xph[:, 1:]

    msel_scale = 2.0 if MODE == "relu" else 1.0
    msel = np.zeros((128, O_PER * 2 * NSTAT + 2 * NSTAT), dtype=np.float32)
    for o in range(O_PER):
        for u in range(NB2):
            for p_half in range(2):
                j = o * 4 + 2 * u + p_half
                col = (o * 2 + u) * NSTAT + j
                msel[p_half * 64 : (p_half + 1) * 64, col] = msel_scale
    # bsel blocks: -1 at row j for all o of this (u, h)
    for u in range(NB2):
        for p_half in range(2):
            for o in range(O_PER):
                j = o * 4 + 2 * u + p_half
                col = O_PER * 2 * NSTAT + u * NSTAT + j
                msel[p_half * 64 : (p_half + 1) * 64, col] = -1.0
    osel = np.zeros((NSTAT, NSTAT), dtype=np.float32)
    for p in range(NSTAT):
        for m in range(NSTAT):
            if p // B == m // B:
                osel[p, m] = 1.0

    in_maps = []
    for core in range(NCORES):
        osl = slice(core * O_PER, (core + 1) * O_PER)
        w = weight[osl]  # [8, 64, 3, 3]
        # wcols[p, o*9+tap] = w[o, p%64, tap//3, tap%3]
        wc = w.reshape(O_PER, C, 9).transpose(1, 0, 2).reshape(C, O_PER * 9)
        wcols = np.concatenate([wc, wc], axis=0).astype(np.float32)  # [128, 72]
        wpack = np.concatenate([wcols, -wcols], axis=1)  # [128, 144]
        # gcol[p] = -gamma[o(p)] with o = p//4 (A = -gamma*rinv)
        gcol = np.repeat(-gamma[osl], B).reshape(NSTAT, 1).astype(np.float32)
        bcol = np.repeat(beta[osl], B).reshape(NSTAT, 1).astype(np.float32)
        spack = np.concatenate([osel, gcol, bcol], axis=1)  # [32, 34]
        in_maps.append(
            {
                "xph": xph,
                "xpho": xpho,
                "wpack": np.ascontiguousarray(wpack),
                "mselh": msel.astype(np.float16),
                "spack": np.ascontiguousarray(spack),
            }
        )
    return in_maps


def _assemble(results):
    out = np.empty((B, O, H, W), dtype=np.float32)
    for core, res in enumerate(results):
        arr = res["out"].reshape(O_PER, B, H, W)  # row = o*4+b
        out[:, core * O_PER : (core + 1) * O_PER] = arr.transpose(1, 0, 2, 3)
    return out


def kernel(x, weight, gamma, beta, _trace=False):
    from concourse import bass_utils

    nc = _build_program()
    in_maps = _host_inputs(x, weight, gamma, beta)
    res = bass_utils.run_bass_kernel_spmd(
        nc, in_maps, core_ids=list(range(NCORES)), trace=_trace
    )
    out = _assemble(res.results)
    if _trace:
        return out, res
    return out
